# revision 19
# baseline (speedup 1.0000x reference)
"""YOLO-detect head (1x1 conv + box decode) on 8 Trainium2 NeuronCores.

Data-parallel over batch: core b processes batch element b.

Per core, per level l (C channels, HW = ny*nx positions):
  p[hw, o] = sum_c x[c, hw] * w[o, c]      (o = a*89 + ch, a anchor, ch channel)
computed on the tensor engine as out = lhsT.T @ rhs with
  lhsT = x chunk  [K=128 channels, M<=128 hw]   (stationary, fp16)
  rhs  = w.T chunk [K=128 channels, N=267]      (moving, fp16)
so the PSUM result is already [hw, 267] — no on-chip transpose.

Decode:
  sigmoid(p) is computed as 0.5*tanh(0.5*p) + 0.5 so that the only ACT table
  set ever needed is exp_and_others (holds BOTH tanh and exp) -> exactly one
  ~2.7us ACT table load for the whole kernel instead of one per
  sigmoid<->exp alternation.
  xy   = sigmoid(p)*stride + grid*stride   (grid*stride precomputed on host)
  wh   = exp(p) * anchor
  rest = sigmoid(p)

DMA regime (what profiling showed matters):
  * The natural (25200, 89) output costs one 356-byte packet per row; instead
    each level is stored as (128, NA, R, 89) — partition p holds rows
    {t*128+p} of each anchor contiguously — and the host transposes back.
  * HWDGE descriptor generation serializes on the issuing sequencer at
    ~0.7us per dma_start, and a blocked store at the head of the FIFO stalls
    every later DMA. So: inputs are host-permuted so each level's x / w loads
    are 1-2 large fully-contiguous-per-partition DMAs on nc.sync, and stores
    go through nc.gpsimd (SWDGE, otherwise-idle queue) so their compute waits
    never block loads.

Inputs x/w are cast to fp16 on host (halves HBM load traffic vs fp32; fp16's
11-bit mantissa + fp32 accumulate keeps the error ~2e-4 of output scale).
"""

import numpy as np

import concourse.bacc as bacc
import concourse.mybir as mybir
import concourse.tile as tile
from concourse.bass_utils import run_bass_kernel_spmd

F32 = mybir.dt.float32
F16 = mybir.dt.float16
AF = mybir.ActivationFunctionType
ALU = mybir.AluOpType

NCORES = 8
NA = 3          # anchors per level
NO = 89         # channels per anchor (80 classes + 5 + 4)
NCOL = NA * NO  # 267
GROUP = 4       # full 128-row hw tiles per PSUM group (4 banks)

LEVELS = [
    dict(C=256,  W=80, HW=6400, stride=8.0,
         anchors=((10.0, 13.0), (16.0, 30.0), (33.0, 23.0))),
    dict(C=512,  W=40, HW=1600, stride=16.0,
         anchors=((30.0, 61.0), (62.0, 45.0), (59.0, 119.0))),
    dict(C=1024, W=20, HW=400,  stride=32.0,
         anchors=((116.0, 90.0), (156.0, 198.0), (373.0, 326.0))),
]


def _ntiles(HW):
    return (HW + 127) // 128


def _groups(HW):
    """[(t0, n_full_tiles, rows_per_tile)]; trailing partial tile is its own group."""
    full, rem = divmod(HW, 128)
    out = []
    t0 = 0
    while t0 < full:
        n = min(GROUP, full - t0)
        out.append((t0, n, 128))
        t0 += n
    if rem:
        out.append((full, 1, rem))
    return out


# levels processed smallest-first: the tiny level-2/1 matmuls start while the
# big level-0 x tensor is still streaming in, and they warm the PE clock gate
ORDER = (2, 1, 0)


def _store_chunks(nt):
    """1-3 store chunks per level (each one anchor-merged DMA)."""
    if nt <= 4 * GROUP:
        return [(0, nt)]
    chunks = []
    s = 0
    while s < nt:
        e = min(s + 4 * GROUP, nt)
        if nt - e <= GROUP:
            e = nt
        chunks.append((s, e - s))
        s = e
    return chunks


def _build_program(use_bias: bool):
    # Bacc (not raw Bass): its compile() runs move_matmul_waits_to_ldweights +
    # generate_event_semaphores, without which walrus rejects instructions
    # that carry more than one semaphore wait.
    nc = bacc.Bacc("TRN2", target_bir_lowering=False, debug=False)

    GSAM_COLS = sum(_ntiles(L["HW"]) * 12 for L in LEVELS)  # 804

    dram = {}
    for l, L in enumerate(LEVELS):
        KC = L["C"] // 128
        nt = _ntiles(L["HW"])
        # x / wt are host-permuted: row p, col (k*HW + w) = x[k*128+p, w]
        dram[f"x{l}"] = nc.dram_tensor(f"x{l}", (128, KC * L["HW"]), F16,
                                       kind="ExternalInput").ap()
        dram[f"wt{l}"] = nc.dram_tensor(f"wt{l}", (128, KC * NCOL), F16,
                                        kind="ExternalInput").ap()
        dram[f"y{l}"] = nc.dram_tensor(f"y{l}", (128, NA, nt, NO), F32,
                                       kind="ExternalOutput").ap()
        if use_bias:
            dram[f"b{l}"] = nc.dram_tensor(f"b{l}", (1, NCOL), F32,
                                           kind="ExternalInput").ap()
    dram["gsam"] = nc.dram_tensor("gsam", (128, GSAM_COLS), F32,
                                  kind="ExternalInput").ap()

    with tile.TileContext(nc) as tc:
        with tc.tile_pool(name="consts", bufs=1) as cpool, \
             tc.tile_pool(name="xbuf", bufs=1) as xpool, \
             tc.tile_pool(name="obuf", bufs=1) as opool, \
             tc.tile_pool(name="ps", bufs=2, space="PSUM") as pspool:

            ones_t = None
            if use_bias:
                ones_t = cpool.tile([1, 128], F16, tag="ones", name="ones")
                nc.vector.memset(ones_t[:, :], 1.0)

            # ---- Phase A: all loads (nc.sync ring carries loads only) ----
            lvl = {}
            for l in ORDER:
                L = LEVELS[l]
                C, HW = L["C"], L["HW"]
                KC = C // 128
                wt_t = cpool.tile([128, KC * NCOL], F16, tag=f"wt{l}",
                                  name=f"wt{l}sb")
                nc.sync.dma_start(out=wt_t[:, :], in_=dram[f"wt{l}"][:, :])

                xk = xpool.tile([128, KC * HW], F16, tag=f"x{l}", name=f"xk{l}")
                if l == 0:
                    # three column-piece DMAs so level-0 matmuls start earlier
                    xs = dram[f"x{l}"].rearrange("p (k w) -> p k w", k=KC)
                    xd = xk.rearrange("p (k w) -> p k w", k=KC)
                    for (c0, c1) in ((0, 1536), (1536, 3200), (3200, HW)):
                        nc.sync.dma_start(out=xd[:, :, c0:c1],
                                          in_=xs[:, :, c0:c1])
                else:
                    nc.sync.dma_start(out=xk[:, :], in_=dram[f"x{l}"][:, :])

                b_t = None
                if use_bias:
                    b_t = cpool.tile([1, NCOL], F16, tag=f"b{l}", name=f"bt{l}")
                    nc.gpsimd.dma_start(out=b_t[:, :], in_=dram[f"b{l}"][:, :])
                lvl[l] = dict(wt=wt_t, xk=xk, b_t=b_t)

                if l == ORDER[0]:
                    gsam_t = cpool.tile([128, GSAM_COLS], F32, tag="gsam",
                                        name="gsamsb")
                    nc.sync.dma_start(out=gsam_t[:, :], in_=dram["gsam"][:, :])

            off = 0
            for l, L in enumerate(LEVELS):
                nt = _ntiles(L["HW"])
                lvl[l]["gs"] = gsam_t[:, off:off + nt * 6].rearrange(
                    "p (t a c) -> p t a c", a=NA, c=2)
                off += nt * 6
                lvl[l]["am"] = gsam_t[:, off:off + nt * 6].rearrange(
                    "p (t a c) -> p t a c", a=NA, c=2)
                off += nt * 6

            # ---- Phase B: compute; stores via SWDGE (gpsimd) ----
            for l in ORDER:
                L = LEVELS[l]
                C, HW, stride = L["C"], L["HW"], L["stride"]
                KC = C // 128
                nt = _ntiles(HW)
                wt_t, xk, b_t = lvl[l]["wt"], lvl[l]["xk"], lvl[l]["b_t"]
                gs_t, am_t = lvl[l]["gs"], lvl[l]["am"]

                # whole level's decoded output stays resident, anchor-major so
                # each (partition, anchor) store run is contiguous; partition p
                # element (a, t, :) is output row hw = t*128+p of anchor a
                ot = opool.tile([128, NA, nt, NO], F32, tag=f"ot{l}", name=f"ot{l}")

                chunks = _store_chunks(nt)
                next_chunk = 0

                for (t0, ntl, m) in _groups(HW):
                    ps = pspool.tile([128, GROUP, 512], F32, tag="ps",
                                     name=f"ps{l}_{t0}")
                    psf = ps.rearrange("p g x -> p (g x)")
                    for i in range(ntl):
                        t = t0 + i
                        for kc in range(KC):
                            nc.tensor.matmul(
                                psf[0:m, i * 512:i * 512 + NCOL],
                                lhsT=xk[:, kc * HW + t * 128:kc * HW + t * 128 + m],
                                rhs=wt_t[:, kc * NCOL:(kc + 1) * NCOL],
                                start=(kc == 0),
                                stop=(kc == KC - 1 and not use_bias),
                            )
                        if use_bias:
                            nc.tensor.matmul(
                                psf[0:m, i * 512:i * 512 + NCOL],
                                lhsT=ones_t[:, 0:m],
                                rhs=b_t[:, :],
                                start=False,
                                stop=True,
                            )

                    og = ot[0:m, :, t0:t0 + ntl, :]  # (m, NA, ntl, 89)
                    # psum viewed anchor-major to match og's enumeration
                    ps_a = ps[0:m, 0:ntl, 0:NCOL].rearrange(
                        "p g (a c) -> p a g c", a=NA)
                    # t = tanh(0.5 * p); sigmoid(p) = 0.5*t + 0.5
                    nc.scalar.activation(og, ps_a, AF.Tanh, scale=0.5)
                    # merged (g c) innermost dim is even -> DVE 2x mode
                    ogf = og.rearrange("p a g c -> p a (g c)")
                    nc.vector.tensor_scalar(ogf, ogf, 1.0, 0.5, ALU.add, ALU.mult)
                    # wh: exp(p) (overwrites the sigmoid values on those cols)
                    nc.scalar.activation(og[:, :, :, 2:4], ps_a[:, :, :, 2:4],
                                         AF.Exp)
                    am_a = am_t[0:m, t0:t0 + ntl].transpose([0, 2, 1, 3])
                    nc.vector.tensor_mul(og[:, :, :, 2:4], og[:, :, :, 2:4], am_a)
                    # xy: sigmoid*stride + grid*stride
                    gs_a = gs_t[0:m, t0:t0 + ntl].transpose([0, 2, 1, 3])
                    og_xy = og[:, :, :, 0:2]
                    nc.vector.tensor_scalar_mul(og_xy, og_xy, float(stride))
                    nc.vector.tensor_add(og_xy, og_xy, gs_a)

                    # emit store chunks whose tile range is now fully decoded
                    while (next_chunk < len(chunks)
                           and chunks[next_chunk][0] + chunks[next_chunk][1]
                           <= t0 + ntl):
                        s0, snt = chunks[next_chunk]
                        nc.gpsimd.dma_start(
                            out=dram[f"y{l}"][:, :, s0:s0 + snt, :],
                            in_=ot[:, :, s0:s0 + snt, :])
                        next_chunk += 1
                assert next_chunk == len(chunks)
    nc.compile()
    return nc


_PROGS = {}


def _get_prog(use_bias: bool):
    if use_bias not in _PROGS:
        _PROGS[use_bias] = _build_program(use_bias)
    return _PROGS[use_bias]


def _host_gsam():
    """Merged [gs0|am0|gs1|am1|gs2|am2] host tensor, (128, 804) fp32."""
    cols = []
    for L in LEVELS:
        HW, W, stride = L["HW"], L["W"], L["stride"]
        nt = _ntiles(HW)
        hw = np.arange(nt * 128)
        gx = (hw % W).astype(np.float32) * stride
        gy = (hw // W).astype(np.float32) * stride
        gx[HW:] = 0.0
        gy[HW:] = 0.0
        gs = np.zeros((128, nt, NA, 2), np.float32)
        gs[:, :, :, 0] = gx.reshape(nt, 128).T[:, :, None]
        gs[:, :, :, 1] = gy.reshape(nt, 128).T[:, :, None]
        am = np.zeros((128, nt, NA, 2), np.float32)
        am[:, :, :, :] = np.asarray(L["anchors"], np.float32)[None, None, :, :]
        cols.append(gs.reshape(128, nt * 6))
        cols.append(am.reshape(128, nt * 6))
    return np.ascontiguousarray(np.concatenate(cols, axis=1))


_CONSTS = None


def _make_in_maps(xs, ws, bs, use_bias):
    global _CONSTS
    if _CONSTS is None:
        _CONSTS = _host_gsam()
    wts, xps = [], []
    for x, w, L in zip(xs, ws, LEVELS):
        KC = L["C"] // 128
        HW = L["HW"]
        # (C, NCOL) -> (128, KC*NCOL): row p col (k*NCOL+o) = w[o, k*128+p]
        wts.append(np.ascontiguousarray(
            w.T.astype(np.float16).reshape(KC, 128, NCOL)
            .transpose(1, 0, 2).reshape(128, KC * NCOL)))
        # (B, C, H, W) -> (B, 128, KC*HW): row p col (k*HW+hw) = x[k*128+p, hw]
        xps.append(np.ascontiguousarray(
            x.reshape(NCORES, KC, 128, HW).astype(np.float16)
            .transpose(0, 2, 1, 3).reshape(NCORES, 128, KC * HW)))
    in_maps = []
    for core in range(NCORES):
        im = {"gsam": _CONSTS}
        for l in range(len(LEVELS)):
            im[f"x{l}"] = xps[l][core]
            im[f"wt{l}"] = wts[l]
            if use_bias:
                im[f"b{l}"] = np.ascontiguousarray(
                    bs[l].reshape(1, NCOL).astype(np.float32))
        in_maps.append(im)
    return in_maps


def _assemble(results):
    """results[core][f"y{l}"] (128, NA, R, 89) -> (NCORES, 25200, 89) fp32."""
    out = np.empty((NCORES, 25200, NO), np.float32)
    for core in range(NCORES):
        parts = []
        for l, L in enumerate(LEVELS):
            HW = L["HW"]
            nt = _ntiles(HW)
            y = results[core][f"y{l}"]  # (128, NA, nt, 89)
            y = y.transpose(1, 2, 0, 3).reshape(NA, nt * 128, NO)[:, :HW, :]
            parts.append(y.reshape(NA * HW, NO))
        out[core] = np.concatenate(parts, axis=0)
    return out


def _run(x0, x1, x2, w0, b0, w1, b1, w2, b2, **spmd_kwargs):
    xs = [np.asarray(x, dtype=np.float32) for x in (x0, x1, x2)]
    ws = [np.asarray(w, dtype=np.float32) for w in (w0, w1, w2)]
    bs = [np.asarray(b, dtype=np.float32) for b in (b0, b1, b2)]
    use_bias = any(np.any(b != 0) for b in bs)
    in_maps = _make_in_maps(xs, ws, bs, use_bias)
    res = run_bass_kernel_spmd(_get_prog(use_bias), in_maps,
                               core_ids=list(range(NCORES)), **spmd_kwargs)
    return _assemble(res.results), res


def kernel(x0, x1, x2, w0, b0, w1, b1, w2, b2):
    out, _ = _run(x0, x1, x2, w0, b0, w1, b1, w2, b2)
    return out


def kernel_traced(x0, x1, x2, w0, b0, w1, b1, w2, b2):
    """Like kernel() but with NTFF tracing; returns (out, BassKernelResults)."""
    return _run(x0, x1, x2, w0, b0, w1, b1, w2, b2, trace=True)


# revision 20
# speedup vs baseline: 1.1676x; 1.1676x over previous
"""YOLO-detect head (1x1 conv + box decode) on 8 Trainium2 NeuronCores.

Data-parallel over batch: core b processes batch element b.

Per core, per level l (C channels, HW = ny*nx positions):
  p[hw, o] = sum_c x[c, hw] * w[o, c]      (o = a*89 + ch, a anchor, ch channel)
computed on the tensor engine as out = lhsT.T @ rhs with
  lhsT = x chunk  [K=128 channels, M<=128 hw]   (stationary, fp16)
  rhs  = w.T chunk [K=128 channels, N=267]      (moving, fp16)
so the PSUM result is already [hw, 267] — no on-chip transpose.

Decode:
  sigmoid(p) is computed as 0.5*tanh(0.5*p) + 0.5 so that the only ACT table
  set ever needed is exp_and_others (holds BOTH tanh and exp) -> exactly one
  ~2.7us ACT table load for the whole kernel instead of one per
  sigmoid<->exp alternation.
  xy   = sigmoid(p)*stride + grid*stride   (grid*stride precomputed on host)
  wh   = exp(p) * anchor
  rest = sigmoid(p)

DMA regime (what profiling showed matters):
  * The natural (25200, 89) output costs one 356-byte packet per row; instead
    each level is stored as (128, NA, R, 89) — partition p holds rows
    {t*128+p} of each anchor contiguously — and the host transposes back.
  * HWDGE descriptor generation serializes on the issuing sequencer at
    ~0.7us per dma_start, and a blocked store at the head of the FIFO stalls
    every later DMA. So: inputs are host-permuted so each level's x / w loads
    are 1-2 large fully-contiguous-per-partition DMAs on nc.sync, and stores
    go through nc.gpsimd (SWDGE, otherwise-idle queue) so their compute waits
    never block loads.

Inputs x/w are cast to fp16 on host (halves HBM load traffic vs fp32; fp16's
11-bit mantissa + fp32 accumulate keeps the error ~2e-4 of output scale).
"""

import numpy as np

import concourse.bacc as bacc
import concourse.mybir as mybir
import concourse.tile as tile
from concourse.bass_utils import run_bass_kernel_spmd

F32 = mybir.dt.float32
F16 = mybir.dt.float16
AF = mybir.ActivationFunctionType
ALU = mybir.AluOpType

NCORES = 8
NA = 3          # anchors per level
NO = 89         # channels per anchor (80 classes + 5 + 4)
NCOL = NA * NO  # 267
GROUP = 2       # full 128-row hw tiles per PSUM group (2 banks)

LEVELS = [
    dict(C=256,  W=80, HW=6400, stride=8.0,
         anchors=((10.0, 13.0), (16.0, 30.0), (33.0, 23.0))),
    dict(C=512,  W=40, HW=1600, stride=16.0,
         anchors=((30.0, 61.0), (62.0, 45.0), (59.0, 119.0))),
    dict(C=1024, W=20, HW=400,  stride=32.0,
         anchors=((116.0, 90.0), (156.0, 198.0), (373.0, 326.0))),
]


def _ntiles(HW):
    return (HW + 127) // 128


def _groups(HW):
    """[(t0, n_full_tiles, rows_per_tile)]; trailing partial tile is its own group."""
    full, rem = divmod(HW, 128)
    out = []
    t0 = 0
    while t0 < full:
        n = min(GROUP, full - t0)
        out.append((t0, n, 128))
        t0 += n
    if rem:
        out.append((full, 1, rem))
    return out


# levels processed smallest-first: the tiny level-2/1 matmuls start while the
# big level-0 x tensor is still streaming in, and they warm the PE clock gate
ORDER = (0, 1, 2)


def _store_chunks(nt):
    """1-3 store chunks per level (each one anchor-merged DMA)."""
    if nt <= 4 * GROUP:
        return [(0, nt)]
    chunks = []
    s = 0
    while s < nt:
        e = min(s + 4 * GROUP, nt)
        if nt - e <= GROUP:
            e = nt
        chunks.append((s, e - s))
        s = e
    return chunks


def _build_program(use_bias: bool):
    # Bacc (not raw Bass): its compile() runs move_matmul_waits_to_ldweights +
    # generate_event_semaphores, without which walrus rejects instructions
    # that carry more than one semaphore wait.
    nc = bacc.Bacc("TRN2", target_bir_lowering=False, debug=False)

    GSAM_COLS = sum(_ntiles(L["HW"]) * 12 for L in LEVELS)  # 804

    dram = {}
    for l, L in enumerate(LEVELS):
        KC = L["C"] // 128
        nt = _ntiles(L["HW"])
        # x / wt are host-permuted: row p, col (k*HW + w) = x[k*128+p, w]
        dram[f"x{l}"] = nc.dram_tensor(f"x{l}", (128, KC * L["HW"]), F16,
                                       kind="ExternalInput").ap()
        dram[f"wt{l}"] = nc.dram_tensor(f"wt{l}", (128, KC * NCOL), F16,
                                        kind="ExternalInput").ap()
        dram[f"y{l}"] = nc.dram_tensor(f"y{l}", (128, NA, nt, NO), F32,
                                       kind="ExternalOutput").ap()
        if use_bias:
            dram[f"b{l}"] = nc.dram_tensor(f"b{l}", (1, NCOL), F32,
                                           kind="ExternalInput").ap()
    dram["gsam"] = nc.dram_tensor("gsam", (128, GSAM_COLS), F32,
                                  kind="ExternalInput").ap()

    with tile.TileContext(nc) as tc:
        with tc.tile_pool(name="consts", bufs=1) as cpool, \
             tc.tile_pool(name="xbuf", bufs=1) as xpool, \
             tc.tile_pool(name="obuf", bufs=1) as opool, \
             tc.tile_pool(name="ps", bufs=4, space="PSUM") as pspool:

            ones_t = None
            if use_bias:
                ones_t = cpool.tile([1, 128], F16, tag="ones", name="ones")
                nc.vector.memset(ones_t[:, :], 1.0)

            # ---- Phase A: all loads (nc.sync ring carries loads only) ----
            lvl = {}
            for l in ORDER:
                L = LEVELS[l]
                C, HW = L["C"], L["HW"]
                KC = C // 128
                wt_t = cpool.tile([128, KC * NCOL], F16, tag=f"wt{l}",
                                  name=f"wt{l}sb")
                nc.sync.dma_start(out=wt_t[:, :], in_=dram[f"wt{l}"][:, :])

                xk = xpool.tile([128, KC * HW], F16, tag=f"x{l}", name=f"xk{l}")
                if l == 0:
                    # three column-piece DMAs so level-0 matmuls start earlier
                    xs = dram[f"x{l}"].rearrange("p (k w) -> p k w", k=KC)
                    xd = xk.rearrange("p (k w) -> p k w", k=KC)
                    for (c0, c1) in ((0, 1536), (1536, 3200), (3200, HW)):
                        nc.sync.dma_start(out=xd[:, :, c0:c1],
                                          in_=xs[:, :, c0:c1])
                else:
                    nc.sync.dma_start(out=xk[:, :], in_=dram[f"x{l}"][:, :])

                b_t = None
                if use_bias:
                    b_t = cpool.tile([1, NCOL], F16, tag=f"b{l}", name=f"bt{l}")
                    nc.gpsimd.dma_start(out=b_t[:, :], in_=dram[f"b{l}"][:, :])
                lvl[l] = dict(wt=wt_t, xk=xk, b_t=b_t)

                if l == ORDER[0]:
                    gsam_t = cpool.tile([128, GSAM_COLS], F32, tag="gsam",
                                        name="gsamsb")
                    nc.sync.dma_start(out=gsam_t[:, :], in_=dram["gsam"][:, :])

            off = 0
            for l, L in enumerate(LEVELS):
                nt = _ntiles(L["HW"])
                lvl[l]["gs"] = gsam_t[:, off:off + nt * 6].rearrange(
                    "p (t a c) -> p t a c", a=NA, c=2)
                off += nt * 6
                lvl[l]["am"] = gsam_t[:, off:off + nt * 6].rearrange(
                    "p (t a c) -> p t a c", a=NA, c=2)
                off += nt * 6

            # ---- Phase B: compute; stores via SWDGE (gpsimd) ----
            for l in ORDER:
                L = LEVELS[l]
                C, HW, stride = L["C"], L["HW"], L["stride"]
                KC = C // 128
                nt = _ntiles(HW)
                wt_t, xk, b_t = lvl[l]["wt"], lvl[l]["xk"], lvl[l]["b_t"]
                gs_t, am_t = lvl[l]["gs"], lvl[l]["am"]

                # whole level's decoded output stays resident, anchor-major so
                # each (partition, anchor) store run is contiguous; partition p
                # element (a, t, :) is output row hw = t*128+p of anchor a
                ot = opool.tile([128, NA, nt, NO], F32, tag=f"ot{l}", name=f"ot{l}")

                chunks = _store_chunks(nt)
                next_chunk = 0

                for (t0, ntl, m) in _groups(HW):
                    ps = pspool.tile([128, GROUP, 512], F32, tag="ps",
                                     name=f"ps{l}_{t0}")
                    psf = ps.rearrange("p g x -> p (g x)")
                    for i in range(ntl):
                        t = t0 + i
                        for kc in range(KC):
                            nc.tensor.matmul(
                                psf[0:m, i * 512:i * 512 + NCOL],
                                lhsT=xk[:, kc * HW + t * 128:kc * HW + t * 128 + m],
                                rhs=wt_t[:, kc * NCOL:(kc + 1) * NCOL],
                                start=(kc == 0),
                                stop=(kc == KC - 1 and not use_bias),
                            )
                        if use_bias:
                            nc.tensor.matmul(
                                psf[0:m, i * 512:i * 512 + NCOL],
                                lhsT=ones_t[:, 0:m],
                                rhs=b_t[:, :],
                                start=False,
                                stop=True,
                            )

                    og = ot[0:m, :, t0:t0 + ntl, :]  # (m, NA, ntl, 89)
                    # psum viewed anchor-major to match og's enumeration
                    ps_a = ps[0:m, 0:ntl, 0:NCOL].rearrange(
                        "p g (a c) -> p a g c", a=NA)
                    # t = tanh(0.5 * p); sigmoid(p) = 0.5*t + 0.5
                    nc.scalar.activation(og, ps_a, AF.Tanh, scale=0.5)
                    # merged (g c) innermost dim is even -> DVE 2x mode
                    ogf = og.rearrange("p a g c -> p a (g c)")
                    nc.vector.tensor_scalar(ogf, ogf, 1.0, 0.5, ALU.add, ALU.mult)
                    # wh: exp(p) (overwrites the sigmoid values on those cols)
                    nc.scalar.activation(og[:, :, :, 2:4], ps_a[:, :, :, 2:4],
                                         AF.Exp)
                    am_a = am_t[0:m, t0:t0 + ntl].transpose([0, 2, 1, 3])
                    nc.vector.tensor_mul(og[:, :, :, 2:4], og[:, :, :, 2:4], am_a)
                    # xy: sigmoid*stride + grid*stride
                    gs_a = gs_t[0:m, t0:t0 + ntl].transpose([0, 2, 1, 3])
                    og_xy = og[:, :, :, 0:2]
                    nc.vector.tensor_scalar_mul(og_xy, og_xy, float(stride))
                    nc.vector.tensor_add(og_xy, og_xy, gs_a)

                    # emit store chunks whose tile range is now fully decoded
                    while (next_chunk < len(chunks)
                           and chunks[next_chunk][0] + chunks[next_chunk][1]
                           <= t0 + ntl):
                        s0, snt = chunks[next_chunk]
                        nc.gpsimd.dma_start(
                            out=dram[f"y{l}"][:, :, s0:s0 + snt, :],
                            in_=ot[:, :, s0:s0 + snt, :])
                        next_chunk += 1
                assert next_chunk == len(chunks)
    nc.compile()
    return nc


_PROGS = {}


def _get_prog(use_bias: bool):
    if use_bias not in _PROGS:
        _PROGS[use_bias] = _build_program(use_bias)
    return _PROGS[use_bias]


def _host_gsam():
    """Merged [gs0|am0|gs1|am1|gs2|am2] host tensor, (128, 804) fp32."""
    cols = []
    for L in LEVELS:
        HW, W, stride = L["HW"], L["W"], L["stride"]
        nt = _ntiles(HW)
        hw = np.arange(nt * 128)
        gx = (hw % W).astype(np.float32) * stride
        gy = (hw // W).astype(np.float32) * stride
        gx[HW:] = 0.0
        gy[HW:] = 0.0
        gs = np.zeros((128, nt, NA, 2), np.float32)
        gs[:, :, :, 0] = gx.reshape(nt, 128).T[:, :, None]
        gs[:, :, :, 1] = gy.reshape(nt, 128).T[:, :, None]
        am = np.zeros((128, nt, NA, 2), np.float32)
        am[:, :, :, :] = np.asarray(L["anchors"], np.float32)[None, None, :, :]
        cols.append(gs.reshape(128, nt * 6))
        cols.append(am.reshape(128, nt * 6))
    return np.ascontiguousarray(np.concatenate(cols, axis=1))


_CONSTS = None


def _make_in_maps(xs, ws, bs, use_bias):
    global _CONSTS
    if _CONSTS is None:
        _CONSTS = _host_gsam()
    wts, xps = [], []
    for x, w, L in zip(xs, ws, LEVELS):
        KC = L["C"] // 128
        HW = L["HW"]
        # (C, NCOL) -> (128, KC*NCOL): row p col (k*NCOL+o) = w[o, k*128+p]
        wts.append(np.ascontiguousarray(
            w.T.astype(np.float16).reshape(KC, 128, NCOL)
            .transpose(1, 0, 2).reshape(128, KC * NCOL)))
        # (B, C, H, W) -> (B, 128, KC*HW): row p col (k*HW+hw) = x[k*128+p, hw]
        xps.append(np.ascontiguousarray(
            x.reshape(NCORES, KC, 128, HW).astype(np.float16)
            .transpose(0, 2, 1, 3).reshape(NCORES, 128, KC * HW)))
    in_maps = []
    for core in range(NCORES):
        im = {"gsam": _CONSTS}
        for l in range(len(LEVELS)):
            im[f"x{l}"] = xps[l][core]
            im[f"wt{l}"] = wts[l]
            if use_bias:
                im[f"b{l}"] = np.ascontiguousarray(
                    bs[l].reshape(1, NCOL).astype(np.float32))
        in_maps.append(im)
    return in_maps


def _assemble(results):
    """results[core][f"y{l}"] (128, NA, R, 89) -> (NCORES, 25200, 89) fp32."""
    out = np.empty((NCORES, 25200, NO), np.float32)
    for core in range(NCORES):
        parts = []
        for l, L in enumerate(LEVELS):
            HW = L["HW"]
            nt = _ntiles(HW)
            y = results[core][f"y{l}"]  # (128, NA, nt, 89)
            y = y.transpose(1, 2, 0, 3).reshape(NA, nt * 128, NO)[:, :HW, :]
            parts.append(y.reshape(NA * HW, NO))
        out[core] = np.concatenate(parts, axis=0)
    return out


def _run(x0, x1, x2, w0, b0, w1, b1, w2, b2, **spmd_kwargs):
    xs = [np.asarray(x, dtype=np.float32) for x in (x0, x1, x2)]
    ws = [np.asarray(w, dtype=np.float32) for w in (w0, w1, w2)]
    bs = [np.asarray(b, dtype=np.float32) for b in (b0, b1, b2)]
    use_bias = any(np.any(b != 0) for b in bs)
    in_maps = _make_in_maps(xs, ws, bs, use_bias)
    res = run_bass_kernel_spmd(_get_prog(use_bias), in_maps,
                               core_ids=list(range(NCORES)), **spmd_kwargs)
    return _assemble(res.results), res


def kernel(x0, x1, x2, w0, b0, w1, b1, w2, b2):
    out, _ = _run(x0, x1, x2, w0, b0, w1, b1, w2, b2)
    return out


def kernel_traced(x0, x1, x2, w0, b0, w1, b1, w2, b2):
    """Like kernel() but with NTFF tracing; returns (out, BassKernelResults)."""
    return _run(x0, x1, x2, w0, b0, w1, b1, w2, b2, trace=True)
